# revision 8
# baseline (speedup 1.0000x reference)
"""GraphSAGE (2-layer, mean aggregation) on 8 Trainium2 NeuronCores.

Strategy (v2 — SBUF-resident feature table):
  - Nodes are sharded contiguously across the 8 cores by destination row.
  - The full feature table (x for layer 1, h for layer 2, 12.8MB bf16)
    lives IN SBUF in a striped layout (row i -> partition i%128, rank
    i//128), so the per-edge gather is an SBUF-source dma_gather
    (transpose mode). This avoids the ~68ns/packet random-256B-HBM-read
    wall that dominated the HBM-gather version.
  - The transposed gather output G^T [feat, edge] feeds a per-chunk
    "transform-first" matmul (linearity: mean(x) @ Wl.T == mean(x@Wl.T)):
    msgs[edge, d_out] = G^T.T @ WlT — which flips edges onto partitions.
  - Scatter per 128-dst block: matmul-accumulate msgs.T @ onehot into
    PSUM, where the 0/1*inv_deg one-hot is built on-chip by the Vector
    engine (is_equal against an iota constant, then * inv_deg values).
    The Wr/bias dense terms accumulate into the same PSUM bank.
  - Hidden states are exchanged between layers with a DRAM AllGather,
    then restriped into the SBUF table with one strided DMA.
  - int16 gather indices can't address 50000 rows, so edges are split
    into lo (src < 32768) / hi streams gathered from two tables.
  - Layer-2 output is written transposed [40, rows]; the host transposes
    it back (index-only reshuffle).
"""

import math
from contextlib import ExitStack

import numpy as np
import ml_dtypes

import concourse.bass as bass
import concourse.bacc as bacc
import concourse.mybir as mybir
import concourse.tile as tile
from concourse import bass_utils

P = 128
N_NODES = 50000
N_EDGES = 800000
D_IN = 128
D_HID = 128
D_OUT = 40
N_CORES = 8
LO_SPLIT = 32768          # int16 gather index limit boundary
RANKS_A = LO_SPLIT // P                        # 256
RANKS_B = math.ceil((N_NODES - LO_SPLIT) / P)  # 135 (17280 slots, 48 pad)
GRP = 16                  # chunks per dma_gather call
GBUFS = 3                 # gather-tile buffering per stream
OBUFS = 3                 # on-chip one-hot tiles in flight
MBUFS = 4                 # msgs tiles in flight
NQ = 4                    # swdge queues

BF16 = ml_dtypes.bfloat16


def _wrap_idxs(idx_flat):
    """dma_gather index layout: idx i lives at [i % 16, i // 16] of a
    16-partition tile, replicated to 128 partitions."""
    n = idx_flat.shape[0]
    assert n % 16 == 0
    w = idx_flat.reshape(n // 16, 16).T.astype(np.int16)  # [16, n/16]
    return np.tile(w, (8, 1))                             # [128, n/16]


def _stripe(rows, nranks):
    """[<=nranks*128, d] row-major -> [128, nranks, d] striped table."""
    pad = nranks * P - rows.shape[0]
    if pad:
        rows = np.concatenate([rows, np.zeros((pad, rows.shape[1]), rows.dtype)])
    return np.ascontiguousarray(
        rows.reshape(nranks, P, rows.shape[1]).transpose(1, 0, 2)
    )


def preprocess(edge_index, n_nodes=N_NODES, n_cores=N_CORES, lo_split=LO_SPLIT):
    """Sort/partition edges; build per-core gather indices + per-chunk dst
    ids and inv-degree values."""
    src = np.asarray(edge_index[0], dtype=np.int64)
    dst = np.asarray(edge_index[1], dtype=np.int64)
    counts = np.bincount(dst, minlength=n_nodes)
    inv_deg = (1.0 / np.maximum(counts, 1)).astype(np.float32)

    rows_per = n_nodes // n_cores
    nblk = math.ceil(rows_per / P)

    order = np.argsort(dst, kind="stable")
    s_s, d_s = src[order], dst[order]

    blk_edges = {}
    n_lo_max, n_hi_max = 0, 0
    for k in range(n_cores):
        base = k * rows_per
        for b in range(nblk):
            r0 = base + b * P
            r1 = min(base + rows_per, r0 + P)
            e0 = np.searchsorted(d_s, r0, side="left")
            e1 = np.searchsorted(d_s, r1, side="left")
            s_seg, d_seg = s_s[e0:e1], d_s[e0:e1]
            lo_m = s_seg < lo_split
            blk_edges[(k, b)] = (s_seg, d_seg, lo_m, r0)
            n_lo_max = max(n_lo_max, int(lo_m.sum()))
            n_hi_max = max(n_hi_max, int((~lo_m).sum()))

    Llo = max(1, math.ceil(n_lo_max / P))
    Lhi = max(1, math.ceil(n_hi_max / P))
    C_lo, C_hi = nblk * Llo, nblk * Lhi

    per_core = []
    for k in range(n_cores):
        idx_lo = np.zeros((C_lo, P), np.int16)
        idx_hi = np.zeros((C_hi, P), np.int16)
        dst_lo = np.full((C_lo, P), -1.0, np.float32)
        dst_hi = np.full((C_hi, P), -1.0, np.float32)
        val_lo = np.zeros((C_lo, P), np.float32)
        val_hi = np.zeros((C_hi, P), np.float32)
        for b in range(nblk):
            s_seg, d_seg, lo_m, r0 = blk_edges[(k, b)]
            for (sel, idx_a, dst_a, val_a, L, off) in (
                (lo_m, idx_lo, dst_lo, val_lo, Llo, 0),
                (~lo_m, idx_hi, dst_hi, val_hi, Lhi, lo_split),
            ):
                ss = s_seg[sel] - off
                dd = d_seg[sel] - r0
                n = ss.shape[0]
                c0 = b * L
                idx_a[c0 : c0 + L].reshape(-1)[:n] = ss.astype(np.int16)
                dst_a[c0 : c0 + L].reshape(-1)[:n] = dd.astype(np.float32)
                val_a[c0 : c0 + L].reshape(-1)[:n] = inv_deg[d_seg[sel]]

        per_core.append(
            dict(
                idx_lo=_wrap_idxs(idx_lo.reshape(-1)),
                idx_hi=_wrap_idxs(idx_hi.reshape(-1)),
                # [128 edge-slot partitions, C chunks]
                dstv_lo=np.ascontiguousarray(dst_lo.T).astype(BF16),
                dstv_hi=np.ascontiguousarray(dst_hi.T).astype(BF16),
                val_lo=np.ascontiguousarray(val_lo.T).astype(BF16),
                val_hi=np.ascontiguousarray(val_hi.T).astype(BF16),
            )
        )

    meta = dict(
        n_nodes=n_nodes, n_cores=n_cores, rows_per=rows_per, nblk=nblk,
        Llo=Llo, Lhi=Lhi, C_lo=C_lo, C_hi=C_hi, lo_split=lo_split,
    )
    return meta, per_core


def build_graph(nc, m, d_in=D_IN, d_out=D_OUT):
    dt = mybir.dt
    alu = mybir.AluOpType
    act = mybir.ActivationFunctionType
    n_nodes, rows_per, nblk = m["n_nodes"], m["rows_per"], m["nblk"]
    Llo, Lhi, C_lo, C_hi = m["Llo"], m["Lhi"], m["C_lo"], m["C_hi"]
    lo_split = m["lo_split"]
    n_pad = (RANKS_A + RANKS_B) * P  # 50176

    xs_lo_d = nc.dram_tensor("xs_lo", [P, RANKS_A, P], dt.bfloat16, kind="ExternalInput")
    xs_hi_d = nc.dram_tensor("xs_hi", [P, RANKS_B, P], dt.bfloat16, kind="ExternalInput")
    xT_d = nc.dram_tensor("xT", [P, rows_per], dt.bfloat16, kind="ExternalInput")
    idx_lo_d = nc.dram_tensor("idx_lo", [P, C_lo * 8], dt.int16, kind="ExternalInput")
    idx_hi_d = nc.dram_tensor("idx_hi", [P, C_hi * 8], dt.int16, kind="ExternalInput")
    dstv_lo_d = nc.dram_tensor("dstv_lo", [P, C_lo], dt.bfloat16, kind="ExternalInput")
    dstv_hi_d = nc.dram_tensor("dstv_hi", [P, C_hi], dt.bfloat16, kind="ExternalInput")
    val_lo_d = nc.dram_tensor("val_lo", [P, C_lo], dt.bfloat16, kind="ExternalInput")
    val_hi_d = nc.dram_tensor("val_hi", [P, C_hi], dt.bfloat16, kind="ExternalInput")
    iota_d = nc.dram_tensor("iota", [P, P], dt.bfloat16, kind="ExternalInput")
    ident_d = nc.dram_tensor("ident", [P, P], dt.bfloat16, kind="ExternalInput")
    w1l_d = nc.dram_tensor("w1lT", [P, d_in], dt.bfloat16, kind="ExternalInput")
    w1r_d = nc.dram_tensor("w1rT", [P, d_in], dt.bfloat16, kind="ExternalInput")
    w2l_d = nc.dram_tensor("w2lT", [P, d_out], dt.bfloat16, kind="ExternalInput")
    w2r_d = nc.dram_tensor("w2rT", [P, d_out], dt.bfloat16, kind="ExternalInput")
    b1_d = nc.dram_tensor("b1r", [1, d_in], dt.bfloat16, kind="ExternalInput")
    b2_d = nc.dram_tensor("b2r", [1, d_out], dt.bfloat16, kind="ExternalInput")
    out_d = nc.dram_tensor("outT", [d_out, rows_per], dt.float32, kind="ExternalOutput")

    with tile.TileContext(nc) as tc, ExitStack() as ctx:
        sb = ctx.enter_context(tc.tile_pool(name="sb", bufs=1))
        dram = ctx.enter_context(tc.tile_pool(name="dram", bufs=1, space="DRAM"))
        psum = ctx.enter_context(tc.tile_pool(name="psum", bufs=1, space="PSUM"))
        glo_p = ctx.enter_context(tc.tile_pool(name="glo", bufs=GBUFS))
        ghi_p = ctx.enter_context(tc.tile_pool(name="ghi", bufs=GBUFS))
        o_p = ctx.enter_context(tc.tile_pool(name="oh", bufs=OBUFS))
        m_p = ctx.enter_context(tc.tile_pool(name="msgs", bufs=MBUFS))
        st_p = ctx.enter_context(tc.tile_pool(name="st", bufs=2))

        def load(shape, dtype, src, name):
            t = sb.tile(shape, dtype, name=name)
            nc.sync.dma_start(t[:], src[:])
            return t

        tabA = load([P, RANKS_A, P], dt.bfloat16, xs_lo_d.ap(), "tabA")
        tabB = load([P, RANKS_B, P], dt.bfloat16, xs_hi_d.ap(), "tabB")
        xT_sb = load([P, rows_per], dt.bfloat16, xT_d.ap(), "xT_sb")
        idxlo_sb = load([P, C_lo * 8], dt.int16, idx_lo_d.ap(), "idxlo_sb")
        idxhi_sb = load([P, C_hi * 8], dt.int16, idx_hi_d.ap(), "idxhi_sb")
        dstlo_sb = load([P, C_lo], dt.bfloat16, dstv_lo_d.ap(), "dstlo_sb")
        dsthi_sb = load([P, C_hi], dt.bfloat16, dstv_hi_d.ap(), "dsthi_sb")
        vallo_sb = load([P, C_lo], dt.bfloat16, val_lo_d.ap(), "vallo_sb")
        valhi_sb = load([P, C_hi], dt.bfloat16, val_hi_d.ap(), "valhi_sb")
        iota_sb = load([P, P], dt.bfloat16, iota_d.ap(), "iota_sb")
        ident_sb = load([P, P], dt.bfloat16, ident_d.ap(), "ident_sb")
        w1l_sb = load([P, d_in], dt.bfloat16, w1l_d.ap(), "w1l_sb")
        w1r_sb = load([P, d_in], dt.bfloat16, w1r_d.ap(), "w1r_sb")
        w2l_sb = load([P, d_out], dt.bfloat16, w2l_d.ap(), "w2l_sb")
        w2r_sb = load([P, d_out], dt.bfloat16, w2r_d.ap(), "w2r_sb")
        b1_sb = load([1, d_in], dt.bfloat16, b1_d.ap(), "b1_sb")
        b2_sb = load([1, d_out], dt.bfloat16, b2_d.ap(), "b2_sb")

        ones_sb = sb.tile([1, 512], dt.bfloat16, name="ones_sb")
        nc.vector.memset(ones_sb[:], 1.0)

        hT = sb.tile([P, rows_per], dt.bfloat16, name="hT")

        hsh = dram.tile([rows_per, d_in], dt.bfloat16, name="hsh")
        hfull = dram.tile([n_pad, d_in], dt.bfloat16, name="hfull")

        qctr = [0]
        cctr = [0]

        def aggregate(outw, wl_sb, wr_sb, bias_sb, rhsT_sb, out_hook):
            """Per dst-block: psum[:outw, dst] = (Wl@mean).scaled + Wr@rhsT
            + bias; out_hook(block, bs, psum) consumes it."""
            streams = {
                "lo": dict(C=C_lo, idx=idxlo_sb, dstv=dstlo_sb, val=vallo_sb,
                           tab=tabA, pool=glo_p, tag="glo"),
                "hi": dict(C=C_hi, idx=idxhi_sb, dstv=dsthi_sb, val=valhi_sb,
                           tab=tabB, pool=ghi_p, tag="ghi"),
            }
            groups = {}
            msgs = {}
            mb = 512 // outw  # chunks per msgs batch

            def ensure_group(stream, g):
                if (stream, g) in groups:
                    return groups[(stream, g)]
                s = streams[stream]
                c0, c1 = g * GRP, min(s["C"], (g + 1) * GRP)
                nch = c1 - c0
                n = nch * P
                t = s["pool"].tile([P, 1, GRP * P], dt.bfloat16, tag=s["tag"],
                                   name=f"g_{s['tag']}")
                nc.gpsimd.dma_gather(
                    t[:, :, :n], s["tab"][:],
                    s["idx"][:, c0 * 8 : c1 * 8],
                    n, n, d_in, transpose=True, single_packet=False,
                    sbuf_tokens_per_rank=P,
                    sbuf_free_dim_per_rank=P * 2,
                    queue_num=0,  # concurrent transposed gathers on different
                                  # queues corrupt each other on HW
                )
                qctr[0] += 1
                ot = o_p.tile([P, GRP, P], dt.bfloat16, tag="ohv", name="ohv")
                nc.vector.tensor_tensor(
                    ot[:, :nch, :],
                    iota_sb[:, None, :].broadcast_to([P, nch, P]),
                    s["dstv"][:, c0:c1, None].broadcast_to([P, nch, P]),
                    alu.is_equal,
                )
                nc.vector.tensor_tensor(
                    ot[:, :nch, :], ot[:, :nch, :],
                    s["val"][:, c0:c1, None].broadcast_to([P, nch, P]),
                    alu.mult,
                )
                groups[(stream, g)] = (t, ot)
                return groups[(stream, g)]

            def ensure_msgs(stream, c):
                b = c // mb
                if (stream, b) in msgs:
                    return msgs[(stream, b)]
                s = streams[stream]
                c0, c1 = b * mb, min(s["C"], (b + 1) * mb)
                mps = psum.tile([P, 512], dt.float32, tag="mps", name="mps",
                                bufs=2)
                for j, cc in enumerate(range(c0, c1)):
                    gt, _ = ensure_group(stream, cc // GRP)
                    e0 = (cc % GRP) * P
                    nc.tensor.matmul(
                        mps[:, j * outw : (j + 1) * outw],
                        lhsT=gt[:, 0, e0 : e0 + P], rhs=wl_sb[:, :outw],
                        start=True, stop=True,
                    )
                mt = m_p.tile([P, 512], dt.bfloat16, tag="msgs", name="msgs")
                w = (c1 - c0) * outw
                if cctr[0] % 2 == 0:
                    nc.vector.tensor_copy(mt[:, :w], mps[:, :w])
                else:
                    nc.scalar.activation(mt[:, :w], mps[:, :w], act.Copy)
                cctr[0] += 1
                msgs[(stream, b)] = mt
                return mt

            for b in range(nblk):
                c0b = b * P
                bs = min(P, rows_per - c0b)
                ps = psum.tile([P, P], dt.float32, tag="agg", name="ps_agg",
                               bufs=4)
                ops = [("lo", c) for c in range(b * Llo, (b + 1) * Llo)]
                ops += [("hi", c) for c in range(b * Lhi, (b + 1) * Lhi)]
                for i, (stream, c) in enumerate(ops):
                    _, ot = ensure_group(stream, c // GRP)
                    mt = ensure_msgs(stream, c)
                    j = c % mb
                    nc.tensor.matmul(
                        ps[:outw, :P],
                        lhsT=mt[:, j * outw : (j + 1) * outw],
                        rhs=ot[:, c % GRP, :],
                        start=(i == 0), stop=False,
                    )
                nc.tensor.matmul(ps[:outw, :bs], lhsT=wr_sb[:, :outw],
                                 rhs=rhsT_sb[:, c0b : c0b + bs],
                                 start=False, stop=False)
                nc.tensor.matmul(ps[:outw, :bs], lhsT=bias_sb[:],
                                 rhs=ones_sb[:, :bs],
                                 start=False, stop=True)
                out_hook(b, bs, ps)

        # ---- layer 1 ----
        def l1_out(b, bs, ps):
            c0 = b * P
            nc.scalar.activation(hT[:, c0 : c0 + bs], ps[:, :bs], act.Relu)
            tr = psum.tile([P, P], dt.bfloat16, tag="tr", name="tr", bufs=2)
            nc.tensor.transpose(tr[:bs, :P], hT[:, c0 : c0 + bs], ident_sb[:])
            hrow = st_p.tile([P, d_in], dt.bfloat16, tag="st", name="hrow")
            nc.vector.tensor_copy(hrow[:bs, :], tr[:bs, :])
            nc.sync.dma_start(hsh[c0 : c0 + bs, :], hrow[:bs, :])

        aggregate(d_in, w1l_sb, w1r_sb, b1_sb, xT_sb, l1_out)

        nc.gpsimd.collective_compute(
            "AllGather", alu.bypass,
            replica_groups=[list(range(m["n_cores"]))],
            ins=[hsh[:].opt()], outs=[hfull[0:n_nodes, :].opt()],
        )

        # restripe h into the SBUF tables for layer-2 gathers
        nc.sync.dma_start(
            tabA[:], hfull[0:lo_split, :].rearrange("(r p) d -> p r d", p=P)
        )
        nc.sync.dma_start(
            tabB[:],
            hfull[lo_split : lo_split + RANKS_B * P, :].rearrange(
                "(r p) d -> p r d", p=P
            ),
        )

        # ---- layer 2 ----
        def l2_out(b, bs, ps):
            c0 = b * P
            ot = st_p.tile([P, P], dt.float32, tag="ot", name="ot")
            nc.vector.tensor_copy(ot[:d_out, :bs], ps[:d_out, :bs])
            nc.sync.dma_start(out_d.ap()[:, c0 : c0 + bs], ot[:d_out, :bs])

        aggregate(d_out, w2l_sb, w2r_sb, b2_sb, hT, l2_out)

    return nc


def make_in_maps(inputs, meta, per_core):
    x = np.asarray(inputs["x"], np.float32)
    n_cores, rows_per = meta["n_cores"], meta["rows_per"]
    lo_split = meta["lo_split"]
    x_bf = x.astype(BF16)
    w1l = np.asarray(inputs["W1l"], np.float32)
    w1r = np.asarray(inputs["W1r"], np.float32)
    w2l = np.asarray(inputs["W2l"], np.float32)
    w2r = np.asarray(inputs["W2r"], np.float32)
    b1 = np.asarray(inputs["b1"], np.float32)
    b2 = np.asarray(inputs["b2"], np.float32)
    iota = np.tile(np.arange(P, dtype=np.float32)[None, :], (P, 1)).astype(BF16)
    ident = np.eye(P, dtype=np.float32).astype(BF16)
    xs_lo = _stripe(x_bf[:lo_split], RANKS_A)
    xs_hi = _stripe(x_bf[lo_split:], RANKS_B)
    in_maps = []
    for k in range(n_cores):
        r0 = k * rows_per
        pc = per_core[k]
        in_maps.append({
            "xs_lo": xs_lo, "xs_hi": xs_hi,
            "xT": np.ascontiguousarray(x[r0 : r0 + rows_per].T).astype(BF16),
            "idx_lo": pc["idx_lo"], "idx_hi": pc["idx_hi"],
            "dstv_lo": pc["dstv_lo"], "dstv_hi": pc["dstv_hi"],
            "val_lo": pc["val_lo"], "val_hi": pc["val_hi"],
            "iota": iota, "ident": ident,
            "w1lT": np.ascontiguousarray(w1l.T).astype(BF16),
            "w1rT": np.ascontiguousarray(w1r.T).astype(BF16),
            "w2lT": np.ascontiguousarray(w2l.T).astype(BF16),
            "w2rT": np.ascontiguousarray(w2r.T).astype(BF16),
            "b1r": b1[None, :].astype(BF16),
            "b2r": b2[None, :].astype(BF16),
        })
    return in_maps


def assemble(res, meta):
    return np.concatenate(
        [np.asarray(res.results[k]["outT"]).T for k in range(meta["n_cores"])],
        axis=0,
    ).astype(np.float32)


_CACHE = {}


def _compile(meta):
    key = (meta["Llo"], meta["Lhi"], meta["n_nodes"], meta["rows_per"])
    if key not in _CACHE:
        nc = bacc.Bacc("TRN2", target_bir_lowering=False, debug=False,
                       num_devices=meta["n_cores"], num_swdge_queues=NQ)
        build_graph(nc, meta)
        nc.compile()
        _CACHE[key] = nc
    return _CACHE[key]


def kernel(**inputs):
    edge_index = np.asarray(inputs["edge_index"])
    meta, per_core = preprocess(edge_index)
    nc = _compile(meta)
    in_maps = make_in_maps(inputs, meta, per_core)
    res = bass_utils.run_bass_kernel_spmd(
        nc, in_maps, core_ids=list(range(meta["n_cores"]))
    )
    return assemble(res, meta)


# revision 11
# speedup vs baseline: 1.2530x; 1.2530x over previous
"""GraphSAGE (2-layer, mean aggregation) on 8 Trainium2 NeuronCores.

Strategy:
  - Nodes are sharded contiguously across the 8 cores by destination row.
  - Aggregation (segment-mean over 800k edges) is done as: dma_gather of
    source-node features onto partitions (128 edges/chunk) and a
    TensorEngine matmul-accumulate into PSUM per 128-dst block, using a
    0/1 one-hot built ON-CHIP by the (otherwise idle) Vector engine via
    is_equal(iota, dst) — this removes ~60MB/core of one-hot HBM reads.
    The 1/deg mean scaling is applied once at PSUM drain time.
  - Hidden states are exchanged between layers with an AllGather
    collective (bf16, row-major) so layer-2 can gather any source row.
  - int16 gather indices can't address 50000 rows, so each block's edges
    are split into lo (src < 32768) and hi streams gathered from two
    slices of the feature table.
"""

import math
from contextlib import ExitStack

import numpy as np
import ml_dtypes

import concourse.bass as bass
import concourse.bacc as bacc
import concourse.mybir as mybir
import concourse.tile as tile
from concourse import bass_utils

P = 128
N_NODES = 50000
N_EDGES = 800000
D_IN = 128
D_HID = 128
D_OUT = 40
N_CORES = 8
LO_SPLIT = 32768          # int16 gather index limit boundary
GRP = 32                  # chunks per dma_gather call
GBUFS = 3                 # gather-tile double/triple buffering per stream
OBUFS = 3                 # on-chip one-hot tiles in flight
NQ = 4                    # swdge queues

BF16 = ml_dtypes.bfloat16


def _wrap_idxs(idx_flat):
    """dma_gather index layout: idx i lives at [i % 16, i // 16] of a
    16-partition tile, replicated to 128 partitions."""
    n = idx_flat.shape[0]
    assert n % 16 == 0
    w = idx_flat.reshape(n // 16, 16).T.astype(np.int16)  # [16, n/16]
    return np.tile(w, (8, 1))                             # [128, n/16]


def preprocess(edge_index, n_nodes=N_NODES, n_cores=N_CORES, lo_split=LO_SPLIT):
    """Sort/partition edges; build per-core gather indices + per-chunk dst ids.

    Returns (meta, per_core) where per_core[k] holds the numpy arrays the
    device kernel consumes and meta holds the (uniform) structure sizes.
    """
    src = np.asarray(edge_index[0], dtype=np.int64)
    dst = np.asarray(edge_index[1], dtype=np.int64)
    counts = np.bincount(dst, minlength=n_nodes)
    inv_deg = (1.0 / np.maximum(counts, 1)).astype(np.float32)

    rows_per = n_nodes // n_cores
    nblk = math.ceil(rows_per / P)

    order = np.argsort(dst, kind="stable")
    s_s, d_s = src[order], dst[order]

    # boundaries of each (core, block) segment in the dst-sorted edge list
    blk_edges = {}
    n_lo_max, n_hi_max = 0, 0
    for k in range(n_cores):
        base = k * rows_per
        for b in range(nblk):
            r0 = base + b * P
            r1 = min(base + rows_per, r0 + P)
            e0 = np.searchsorted(d_s, r0, side="left")
            e1 = np.searchsorted(d_s, r1, side="left")
            s_seg, d_seg = s_s[e0:e1], d_s[e0:e1]
            lo_m = s_seg < lo_split
            blk_edges[(k, b)] = (s_seg, d_seg, lo_m, r0)
            n_lo_max = max(n_lo_max, int(lo_m.sum()))
            n_hi_max = max(n_hi_max, int((~lo_m).sum()))

    Llo = max(1, math.ceil(n_lo_max / P))
    Lhi = max(1, math.ceil(n_hi_max / P))
    C_lo, C_hi = nblk * Llo, nblk * Lhi

    per_core = []
    for k in range(n_cores):
        idx_lo = np.zeros((C_lo, P), np.int16)
        idx_hi = np.zeros((C_hi, P), np.int16)
        dst_lo = np.full((C_lo, P), -1.0, np.float32)
        dst_hi = np.full((C_hi, P), -1.0, np.float32)
        for b in range(nblk):
            s_seg, d_seg, lo_m, r0 = blk_edges[(k, b)]
            for (sel, idx_a, dst_a, L, off) in (
                (lo_m, idx_lo, dst_lo, Llo, 0),
                (~lo_m, idx_hi, dst_hi, Lhi, lo_split),
            ):
                ss = s_seg[sel] - off
                dd = d_seg[sel] - r0
                n = ss.shape[0]
                c0 = b * L
                fl_i = idx_a[c0 : c0 + L].reshape(-1)
                fl_d = dst_a[c0 : c0 + L].reshape(-1)
                fl_i[:n] = ss.astype(np.int16)
                fl_d[:n] = dd.astype(np.float32)

        r0 = k * rows_per
        per_core.append(
            dict(
                idx_lo=_wrap_idxs(idx_lo.reshape(-1)),
                idx_hi=_wrap_idxs(idx_hi.reshape(-1)),
                # [128 edge-slot partitions, C chunks]
                dstv_lo=np.ascontiguousarray(dst_lo.T).astype(BF16),
                dstv_hi=np.ascontiguousarray(dst_hi.T).astype(BF16),
                invdeg=np.tile(
                    inv_deg[r0 : r0 + rows_per][None, :], (P, 1)
                ).astype(BF16),
            )
        )

    meta = dict(
        n_nodes=n_nodes, n_cores=n_cores, rows_per=rows_per, nblk=nblk,
        Llo=Llo, Lhi=Lhi, C_lo=C_lo, C_hi=C_hi, lo_split=lo_split,
    )
    return meta, per_core


def build_graph(nc, m, d_in=D_IN, d_out=D_OUT):
    dt = mybir.dt
    alu = mybir.AluOpType
    act = mybir.ActivationFunctionType
    n_nodes, rows_per, nblk = m["n_nodes"], m["rows_per"], m["nblk"]
    Llo, Lhi, C_lo, C_hi = m["Llo"], m["Lhi"], m["C_lo"], m["C_hi"]
    lo_split = m["lo_split"]

    x_all = nc.dram_tensor("x_all", [n_nodes, d_in], dt.bfloat16, kind="ExternalInput")
    xT_d = nc.dram_tensor("xT", [P, rows_per], dt.bfloat16, kind="ExternalInput")
    idx_lo_d = nc.dram_tensor("idx_lo", [P, C_lo * 8], dt.int16, kind="ExternalInput")
    idx_hi_d = nc.dram_tensor("idx_hi", [P, C_hi * 8], dt.int16, kind="ExternalInput")
    dstv_lo_d = nc.dram_tensor("dstv_lo", [P, C_lo], dt.bfloat16, kind="ExternalInput")
    dstv_hi_d = nc.dram_tensor("dstv_hi", [P, C_hi], dt.bfloat16, kind="ExternalInput")
    invdeg_d = nc.dram_tensor("invdeg", [P, rows_per], dt.bfloat16, kind="ExternalInput")
    iota_d = nc.dram_tensor("iota", [P, P], dt.bfloat16, kind="ExternalInput")
    w1l_d = nc.dram_tensor("w1lT", [P, d_in], dt.bfloat16, kind="ExternalInput")
    w1r_d = nc.dram_tensor("w1rT", [P, d_in], dt.bfloat16, kind="ExternalInput")
    w2l_d = nc.dram_tensor("w2lT", [P, d_out], dt.bfloat16, kind="ExternalInput")
    w2r_d = nc.dram_tensor("w2rT", [P, d_out], dt.bfloat16, kind="ExternalInput")
    b1_d = nc.dram_tensor("b1r", [1, d_in], dt.bfloat16, kind="ExternalInput")
    b2_d = nc.dram_tensor("b2r", [1, d_out], dt.bfloat16, kind="ExternalInput")
    out_d = nc.dram_tensor("out", [rows_per, d_out], dt.float32, kind="ExternalOutput")

    with tile.TileContext(nc) as tc, ExitStack() as ctx:
        sb = ctx.enter_context(tc.tile_pool(name="sb", bufs=1))
        dram = ctx.enter_context(tc.tile_pool(name="dram", bufs=1, space="DRAM"))
        psum = ctx.enter_context(tc.tile_pool(name="psum", bufs=1, space="PSUM"))
        glo_p = ctx.enter_context(tc.tile_pool(name="glo", bufs=GBUFS))
        ghi_p = ctx.enter_context(tc.tile_pool(name="ghi", bufs=GBUFS))
        o_p = ctx.enter_context(tc.tile_pool(name="oh", bufs=OBUFS))
        st_p = ctx.enter_context(tc.tile_pool(name="st", bufs=2))

        def load(shape, dtype, src, name):
            t = sb.tile(shape, dtype, name=name)
            nc.sync.dma_start(t[:], src[:])
            return t

        xT_sb = load([P, rows_per], dt.bfloat16, xT_d.ap(), "xT_sb")
        idxlo_sb = load([P, C_lo * 8], dt.int16, idx_lo_d.ap(), "idxlo_sb")
        idxhi_sb = load([P, C_hi * 8], dt.int16, idx_hi_d.ap(), "idxhi_sb")
        dstlo_sb = load([P, C_lo], dt.bfloat16, dstv_lo_d.ap(), "dstlo_sb")
        dsthi_sb = load([P, C_hi], dt.bfloat16, dstv_hi_d.ap(), "dsthi_sb")
        invdeg_sb = load([P, rows_per], dt.bfloat16, invdeg_d.ap(), "invdeg_sb")
        iota_sb = load([P, P], dt.bfloat16, iota_d.ap(), "iota_sb")
        w1l_sb = load([P, d_in], dt.bfloat16, w1l_d.ap(), "w1l_sb")
        w1r_sb = load([P, d_in], dt.bfloat16, w1r_d.ap(), "w1r_sb")
        w2l_sb = load([P, d_out], dt.bfloat16, w2l_d.ap(), "w2l_sb")
        w2r_sb = load([P, d_out], dt.bfloat16, w2r_d.ap(), "w2r_sb")
        b1_sb = load([1, d_in], dt.bfloat16, b1_d.ap(), "b1_sb")
        b2_sb = load([1, d_out], dt.bfloat16, b2_d.ap(), "b2_sb")

        ones_sb = sb.tile([1, 512], dt.bfloat16, name="ones_sb")
        nc.vector.memset(ones_sb[:], 1.0)

        meanT = sb.tile([P, rows_per], dt.bfloat16, name="meanT")
        meanhT = sb.tile([P, rows_per], dt.bfloat16, name="meanhT")
        hT = sb.tile([P, rows_per], dt.bfloat16, name="hT")

        hsh = dram.tile([rows_per, d_in], dt.bfloat16, name="hsh")
        hfull = dram.tile([n_nodes, d_in], dt.bfloat16, name="hfull")

        qctr = [0]

        def aggregate(src_ap, outT):
            """outT[:, i] = (1/deg(i)) * sum_e src[srcnode(e), :] over edges
            into i. src rows gathered per edge; one-hot built on-chip."""
            streams = {
                "lo": dict(C=C_lo, idx=idxlo_sb, dstv=dstlo_sb,
                           ap=src_ap[0:lo_split, :], pool=glo_p, tag="glo"),
                "hi": dict(C=C_hi, idx=idxhi_sb, dstv=dsthi_sb,
                           ap=src_ap[lo_split:n_nodes, :], pool=ghi_p, tag="ghi"),
            }
            tiles = {}

            def ensure_group(stream, g):
                if (stream, g) in tiles:
                    return tiles[(stream, g)]
                s = streams[stream]
                c0, c1 = g * GRP, min(s["C"], (g + 1) * GRP)
                nch = c1 - c0
                n = nch * P
                t = s["pool"].tile([P, GRP, P], dt.bfloat16, tag=s["tag"],
                                   name=f"g_{s['tag']}")
                nc.gpsimd.dma_gather(
                    t[:, :nch, :], s["ap"],
                    s["idx"][:, c0 * 8 : c1 * 8],
                    n, n, d_in, elem_step=d_in, single_packet=False,
                    queue_num=qctr[0] % NQ,
                )
                qctr[0] += 1
                ot = o_p.tile([P, GRP, P], dt.bfloat16, tag="ohv", name="ohv")
                nc.vector.tensor_tensor(
                    ot[:, :nch, :],
                    iota_sb[:, None, :].broadcast_to([P, nch, P]),
                    s["dstv"][:, c0:c1, None].broadcast_to([P, nch, P]),
                    alu.is_equal,
                )
                tiles[(stream, g)] = (t, ot)
                return tiles[(stream, g)]

            for b in range(nblk):
                bs = min(P, rows_per - b * P)
                ps = psum.tile([P, P], dt.float32, tag="agg", name="ps_agg",
                               bufs=4)
                ops = [("lo", c) for c in range(b * Llo, (b + 1) * Llo)]
                ops += [("hi", c) for c in range(b * Lhi, (b + 1) * Lhi)]
                for i, (stream, c) in enumerate(ops):
                    gt, ot = ensure_group(stream, c // GRP)
                    nc.tensor.matmul(
                        ps[:, :P], lhsT=gt[:, c % GRP, :], rhs=ot[:, c % GRP, :],
                        start=(i == 0), stop=(i == len(ops) - 1),
                    )
                nc.vector.tensor_tensor(
                    outT[:, b * P : b * P + bs], ps[:, :bs],
                    invdeg_sb[:, b * P : b * P + bs], alu.mult,
                )

        # ---- layer 1 ----
        aggregate(x_all.ap(), meanT)

        # row-major h (for the collective) first so the AllGather can start
        # while the hT panels below still run.
        for b in range(nblk):
            c0 = b * P
            bs = min(P, rows_per - c0)
            ps = psum.tile([P, 512], dt.float32, tag="ps", name="ps_r", bufs=4)
            nc.tensor.matmul(ps[:bs, :d_in], lhsT=meanT[:, c0 : c0 + bs], rhs=w1l_sb[:],
                             start=True, stop=False)
            nc.tensor.matmul(ps[:bs, :d_in], lhsT=xT_sb[:, c0 : c0 + bs], rhs=w1r_sb[:],
                             start=False, stop=False)
            nc.tensor.matmul(ps[:bs, :d_in], lhsT=ones_sb[:, :bs], rhs=b1_sb[:],
                             start=False, stop=True)
            hrow = st_p.tile([P, d_in], dt.bfloat16, tag="st", name="hrow")
            nc.scalar.activation(hrow[:bs, :], ps[:bs, :d_in], act.Relu)
            nc.sync.dma_start(hsh[c0 : c0 + bs, :], hrow[:bs, :])

        nc.gpsimd.collective_compute(
            "AllGather", alu.bypass,
            replica_groups=[list(range(m["n_cores"]))],
            ins=[hsh[:].opt()], outs=[hfull[:].opt()],
        )

        for c0 in range(0, rows_per, 512):
            w = min(512, rows_per - c0)
            ps = psum.tile([P, 512], dt.float32, tag="ps", name="ps_d", bufs=4)
            nc.tensor.matmul(ps[:, :w], lhsT=w1l_sb[:], rhs=meanT[:, c0 : c0 + w],
                             start=True, stop=False)
            nc.tensor.matmul(ps[:, :w], lhsT=w1r_sb[:], rhs=xT_sb[:, c0 : c0 + w],
                             start=False, stop=False)
            nc.tensor.matmul(ps[:, :w], lhsT=b1_sb[:], rhs=ones_sb[:, :w],
                             start=False, stop=True)
            nc.scalar.activation(hT[:, c0 : c0 + w], ps[:, :w], act.Relu)

        # ---- layer 2 ----
        aggregate(hfull, meanhT)

        for b in range(nblk):
            c0 = b * P
            bs = min(P, rows_per - c0)
            ps = psum.tile([P, 512], dt.float32, tag="ps", name="ps_o", bufs=4)
            nc.tensor.matmul(ps[:bs, :d_out], lhsT=meanhT[:, c0 : c0 + bs], rhs=w2l_sb[:],
                             start=True, stop=False)
            nc.tensor.matmul(ps[:bs, :d_out], lhsT=hT[:, c0 : c0 + bs], rhs=w2r_sb[:],
                             start=False, stop=False)
            nc.tensor.matmul(ps[:bs, :d_out], lhsT=ones_sb[:, :bs], rhs=b2_sb[:],
                             start=False, stop=True)
            ot = st_p.tile([P, d_out], dt.float32, tag="ot", name="ot")
            nc.vector.tensor_copy(ot[:bs, :], ps[:bs, :d_out])
            nc.sync.dma_start(out_d.ap()[c0 : c0 + bs, :], ot[:bs, :])

    return nc


def make_in_maps(inputs, meta, per_core):
    x = np.asarray(inputs["x"], np.float32)
    n_cores, rows_per = meta["n_cores"], meta["rows_per"]
    x_bf = x.astype(BF16)
    w1l = np.asarray(inputs["W1l"], np.float32)
    w1r = np.asarray(inputs["W1r"], np.float32)
    w2l = np.asarray(inputs["W2l"], np.float32)
    w2r = np.asarray(inputs["W2r"], np.float32)
    b1 = np.asarray(inputs["b1"], np.float32)
    b2 = np.asarray(inputs["b2"], np.float32)
    iota = np.tile(np.arange(P, dtype=np.float32)[None, :], (P, 1)).astype(BF16)
    in_maps = []
    for k in range(n_cores):
        r0 = k * rows_per
        pc = per_core[k]
        in_maps.append({
            "x_all": x_bf,
            "xT": np.ascontiguousarray(x[r0 : r0 + rows_per].T).astype(BF16),
            "idx_lo": pc["idx_lo"], "idx_hi": pc["idx_hi"],
            "dstv_lo": pc["dstv_lo"], "dstv_hi": pc["dstv_hi"],
            "invdeg": pc["invdeg"],
            "iota": iota,
            "w1lT": np.ascontiguousarray(w1l.T).astype(BF16),
            "w1rT": np.ascontiguousarray(w1r.T).astype(BF16),
            "w2lT": np.ascontiguousarray(w2l.T).astype(BF16),
            "w2rT": np.ascontiguousarray(w2r.T).astype(BF16),
            "b1r": b1[None, :].astype(BF16),
            "b2r": b2[None, :].astype(BF16),
        })
    return in_maps


_CACHE = {}


def _compile(meta):
    key = (meta["Llo"], meta["Lhi"], meta["n_nodes"], meta["rows_per"])
    if key not in _CACHE:
        nc = bacc.Bacc("TRN2", target_bir_lowering=False, debug=False,
                       num_devices=meta["n_cores"], num_swdge_queues=NQ,
                       dynamic_dma_scratch_size=49152)
        build_graph(nc, meta)
        nc.compile()
        _CACHE[key] = nc
    return _CACHE[key]


def kernel(**inputs):
    edge_index = np.asarray(inputs["edge_index"])
    meta, per_core = preprocess(edge_index)
    nc = _compile(meta)
    in_maps = make_in_maps(inputs, meta, per_core)
    res = bass_utils.run_bass_kernel_spmd(
        nc, in_maps, core_ids=list(range(meta["n_cores"]))
    )
    out = np.concatenate(
        [res.results[k]["out"] for k in range(meta["n_cores"])], axis=0
    )
    return out.astype(np.float32)


# revision 13
# speedup vs baseline: 1.4918x; 1.1906x over previous
"""GraphSAGE (2-layer, mean aggregation) on 8 Trainium2 NeuronCores.

Strategy:
  - Nodes are sharded contiguously across the 8 cores by destination row.
  - Aggregation (segment-mean over 800k edges) is done as: dma_gather of
    source-node features onto partitions (128 edges/chunk) and a
    TensorEngine matmul-accumulate into PSUM per 128-dst block, using a
    0/1 one-hot built ON-CHIP by the (otherwise idle) Vector engine via
    is_equal(iota, dst) — this removes ~60MB/core of one-hot HBM reads.
    The 1/deg mean scaling is applied once at PSUM drain time.
  - Hidden states are exchanged between layers with an AllGather
    collective (bf16, row-major) so layer-2 can gather any source row.
  - int16 gather indices can't address 50000 rows, so each block's edges
    are split into lo (src < 32768) and hi streams gathered from two
    slices of the feature table.
"""

import math
from contextlib import ExitStack

import numpy as np
import ml_dtypes

import concourse.bass as bass
import concourse.bacc as bacc
import concourse.mybir as mybir
import concourse.tile as tile
from concourse import bass_utils

P = 128
N_NODES = 50000
N_EDGES = 800000
D_IN = 128
D_HID = 128
D_OUT = 40
N_CORES = 8
LO_SPLIT = 32768          # int16 gather index limit boundary
GRP = 32                  # chunks per dma_gather call
GBUFS = 3                 # gather-tile double/triple buffering per stream
OBUFS = 3                 # on-chip one-hot tiles in flight
NQ = 4                    # swdge queues

BF16 = ml_dtypes.bfloat16


def _wrap_idxs(idx_flat):
    """dma_gather index layout: idx i lives at [i % 16, i // 16] of a
    16-partition tile, replicated to 128 partitions."""
    n = idx_flat.shape[0]
    assert n % 16 == 0
    w = idx_flat.reshape(n // 16, 16).T.astype(np.int16)  # [16, n/16]
    return np.tile(w, (8, 1))                             # [128, n/16]


def preprocess(edge_index, n_nodes=N_NODES, n_cores=N_CORES, lo_split=LO_SPLIT):
    """Sort/partition edges; build per-core gather indices + per-chunk dst ids.

    Returns (meta, per_core) where per_core[k] holds the numpy arrays the
    device kernel consumes and meta holds the (uniform) structure sizes.
    """
    src = np.asarray(edge_index[0], dtype=np.int64)
    dst = np.asarray(edge_index[1], dtype=np.int64)
    counts = np.bincount(dst, minlength=n_nodes)
    inv_deg = (1.0 / np.maximum(counts, 1)).astype(np.float32)

    rows_per = n_nodes // n_cores
    nblk = math.ceil(rows_per / P)

    order = np.argsort(dst, kind="stable")
    s_s, d_s = src[order], dst[order]

    # boundaries of each (core, block) segment in the dst-sorted edge list
    blk_edges = {}
    n_lo_max, n_hi_max = 0, 0
    for k in range(n_cores):
        base = k * rows_per
        for b in range(nblk):
            r0 = base + b * P
            r1 = min(base + rows_per, r0 + P)
            e0 = np.searchsorted(d_s, r0, side="left")
            e1 = np.searchsorted(d_s, r1, side="left")
            s_seg, d_seg = s_s[e0:e1], d_s[e0:e1]
            lo_m = s_seg < lo_split
            blk_edges[(k, b)] = (s_seg, d_seg, lo_m, r0)
            n_lo_max = max(n_lo_max, int(lo_m.sum()))
            n_hi_max = max(n_hi_max, int((~lo_m).sum()))

    Llo = max(1, math.ceil(n_lo_max / P))
    Lhi = max(1, math.ceil(n_hi_max / P))
    C_lo, C_hi = nblk * Llo, nblk * Lhi

    per_core = []
    for k in range(n_cores):
        idx_lo = np.zeros((C_lo, P), np.int16)
        idx_hi = np.zeros((C_hi, P), np.int16)
        dst_lo = np.full((C_lo, P), -1.0, np.float32)
        dst_hi = np.full((C_hi, P), -1.0, np.float32)
        for b in range(nblk):
            s_seg, d_seg, lo_m, r0 = blk_edges[(k, b)]
            for (sel, idx_a, dst_a, L, off) in (
                (lo_m, idx_lo, dst_lo, Llo, 0),
                (~lo_m, idx_hi, dst_hi, Lhi, lo_split),
            ):
                ss = s_seg[sel] - off
                dd = d_seg[sel] - r0
                n = ss.shape[0]
                c0 = b * L
                fl_i = idx_a[c0 : c0 + L].reshape(-1)
                fl_d = dst_a[c0 : c0 + L].reshape(-1)
                fl_i[:n] = ss.astype(np.int16)
                fl_d[:n] = dd.astype(np.float32)

        r0 = k * rows_per
        per_core.append(
            dict(
                idx_lo=_wrap_idxs(idx_lo.reshape(-1)),
                idx_hi=_wrap_idxs(idx_hi.reshape(-1)),
                # [128 edge-slot partitions, C chunks]
                dstv_lo=np.ascontiguousarray(dst_lo.T).astype(BF16),
                dstv_hi=np.ascontiguousarray(dst_hi.T).astype(BF16),
                invdeg=np.tile(
                    inv_deg[r0 : r0 + rows_per][None, :], (P, 1)
                ).astype(BF16),
            )
        )

    meta = dict(
        n_nodes=n_nodes, n_cores=n_cores, rows_per=rows_per, nblk=nblk,
        Llo=Llo, Lhi=Lhi, C_lo=C_lo, C_hi=C_hi, lo_split=lo_split,
    )
    return meta, per_core


def build_graph(nc, m, d_in=D_IN, d_out=D_OUT):
    dt = mybir.dt
    alu = mybir.AluOpType
    act = mybir.ActivationFunctionType
    n_nodes, rows_per, nblk = m["n_nodes"], m["rows_per"], m["nblk"]
    Llo, Lhi, C_lo, C_hi = m["Llo"], m["Lhi"], m["C_lo"], m["C_hi"]
    lo_split = m["lo_split"]

    x_all = nc.dram_tensor("x_all", [n_nodes, d_in], dt.bfloat16, kind="ExternalInput")
    xT_d = nc.dram_tensor("xT", [P, rows_per], dt.bfloat16, kind="ExternalInput")
    idx_lo_d = nc.dram_tensor("idx_lo", [P, C_lo * 8], dt.int16, kind="ExternalInput")
    idx_hi_d = nc.dram_tensor("idx_hi", [P, C_hi * 8], dt.int16, kind="ExternalInput")
    dstv_lo_d = nc.dram_tensor("dstv_lo", [P, C_lo], dt.bfloat16, kind="ExternalInput")
    dstv_hi_d = nc.dram_tensor("dstv_hi", [P, C_hi], dt.bfloat16, kind="ExternalInput")
    invdeg_d = nc.dram_tensor("invdeg", [P, rows_per], dt.bfloat16, kind="ExternalInput")
    iota_d = nc.dram_tensor("iota", [P, P], dt.bfloat16, kind="ExternalInput")
    w1l_d = nc.dram_tensor("w1lT", [P, d_in], dt.bfloat16, kind="ExternalInput")
    w1r_d = nc.dram_tensor("w1rT", [P, d_in], dt.bfloat16, kind="ExternalInput")
    w2l_d = nc.dram_tensor("w2lT", [P, d_out], dt.bfloat16, kind="ExternalInput")
    w2r_d = nc.dram_tensor("w2rT", [P, d_out], dt.bfloat16, kind="ExternalInput")
    b1_d = nc.dram_tensor("b1r", [1, d_in], dt.bfloat16, kind="ExternalInput")
    b2_d = nc.dram_tensor("b2r", [1, d_out], dt.bfloat16, kind="ExternalInput")
    out_d = nc.dram_tensor("out", [rows_per, d_out], dt.float32, kind="ExternalOutput")

    with tile.TileContext(nc) as tc, ExitStack() as ctx:
        sb = ctx.enter_context(tc.tile_pool(name="sb", bufs=1))
        dram = ctx.enter_context(tc.tile_pool(name="dram", bufs=1, space="DRAM"))
        psum = ctx.enter_context(tc.tile_pool(name="psum", bufs=1, space="PSUM"))
        glo_p = ctx.enter_context(tc.tile_pool(name="glo", bufs=GBUFS))
        ghi_p = ctx.enter_context(tc.tile_pool(name="ghi", bufs=GBUFS))
        o_p = ctx.enter_context(tc.tile_pool(name="oh", bufs=OBUFS))
        st_p = ctx.enter_context(tc.tile_pool(name="st", bufs=2))

        def load(shape, dtype, src, name):
            t = sb.tile(shape, dtype, name=name)
            nc.sync.dma_start(t[:], src[:])
            return t

        xT_sb = load([P, rows_per], dt.bfloat16, xT_d.ap(), "xT_sb")
        idxlo_sb = load([P, C_lo * 8], dt.int16, idx_lo_d.ap(), "idxlo_sb")
        idxhi_sb = load([P, C_hi * 8], dt.int16, idx_hi_d.ap(), "idxhi_sb")
        dstlo_sb = load([P, C_lo], dt.bfloat16, dstv_lo_d.ap(), "dstlo_sb")
        dsthi_sb = load([P, C_hi], dt.bfloat16, dstv_hi_d.ap(), "dsthi_sb")
        invdeg_sb = load([P, rows_per], dt.bfloat16, invdeg_d.ap(), "invdeg_sb")
        iota_sb = load([P, P], dt.bfloat16, iota_d.ap(), "iota_sb")
        w1l_sb = load([P, d_in], dt.bfloat16, w1l_d.ap(), "w1l_sb")
        w1r_sb = load([P, d_in], dt.bfloat16, w1r_d.ap(), "w1r_sb")
        w2l_sb = load([P, d_out], dt.bfloat16, w2l_d.ap(), "w2l_sb")
        w2r_sb = load([P, d_out], dt.bfloat16, w2r_d.ap(), "w2r_sb")
        b1_sb = load([1, d_in], dt.bfloat16, b1_d.ap(), "b1_sb")
        b2_sb = load([1, d_out], dt.bfloat16, b2_d.ap(), "b2_sb")

        ones_sb = sb.tile([1, 512], dt.bfloat16, name="ones_sb")
        nc.vector.memset(ones_sb[:], 1.0)

        meanT = sb.tile([P, rows_per], dt.bfloat16, name="meanT")
        meanhT = sb.tile([P, rows_per], dt.bfloat16, name="meanhT")
        hT = sb.tile([P, rows_per], dt.bfloat16, name="hT")

        hsh = dram.tile([rows_per, d_in], dt.bfloat16, name="hsh")
        hfull = dram.tile([n_nodes, d_in], dt.bfloat16, name="hfull")

        qctr = [0]

        def aggregate(src_ap, outT):
            """outT[:, i] = (1/deg(i)) * sum_e src[srcnode(e), :] over edges
            into i. src rows gathered per edge; one-hot built on-chip."""
            streams = {
                "lo": dict(C=C_lo, idx=idxlo_sb, dstv=dstlo_sb,
                           ap=src_ap[0:lo_split, :], pool=glo_p, tag="glo"),
                "hi": dict(C=C_hi, idx=idxhi_sb, dstv=dsthi_sb,
                           ap=src_ap[lo_split:n_nodes, :], pool=ghi_p, tag="ghi"),
            }
            tiles = {}

            def ensure_group(stream, g):
                if (stream, g) in tiles:
                    return tiles[(stream, g)]
                s = streams[stream]
                c0, c1 = g * GRP, min(s["C"], (g + 1) * GRP)
                nch = c1 - c0
                n = nch * P
                t = s["pool"].tile([P, GRP, P], dt.bfloat16, tag=s["tag"],
                                   name=f"g_{s['tag']}")
                nc.gpsimd.dma_gather(
                    t[:, :nch, :], s["ap"],
                    s["idx"][:, c0 * 8 : c1 * 8],
                    n, n, d_in, elem_step=d_in, single_packet=False,
                    queue_num=qctr[0] % NQ,
                )
                qctr[0] += 1
                ot = o_p.tile([P, GRP, P], dt.bfloat16, tag="ohv", name="ohv")
                nc.vector.tensor_tensor(
                    ot[:, :nch, :],
                    iota_sb[:, None, :].broadcast_to([P, nch, P]),
                    s["dstv"][:, c0:c1, None].broadcast_to([P, nch, P]),
                    alu.is_equal,
                )
                tiles[(stream, g)] = (t, ot)
                return tiles[(stream, g)]

            for b in range(nblk):
                bs = min(P, rows_per - b * P)
                ps = psum.tile([P, P], dt.float32, tag="agg", name="ps_agg",
                               bufs=4)
                ops = [("lo", c) for c in range(b * Llo, (b + 1) * Llo)]
                ops += [("hi", c) for c in range(b * Lhi, (b + 1) * Lhi)]
                for i, (stream, c) in enumerate(ops):
                    gt, ot = ensure_group(stream, c // GRP)
                    nc.tensor.matmul(
                        ps[:, :P], lhsT=gt[:, c % GRP, :], rhs=ot[:, c % GRP, :],
                        start=(i == 0), stop=(i == len(ops) - 1),
                    )
                nc.vector.tensor_tensor(
                    outT[:, b * P : b * P + bs], ps[:, :bs],
                    invdeg_sb[:, b * P : b * P + bs], alu.mult,
                )

        # ---- layer 1 ----
        aggregate(x_all.ap(), meanT)

        # row-major h (for the collective) first so the AllGather can start
        # while the hT panels below still run.
        for b in range(nblk):
            c0 = b * P
            bs = min(P, rows_per - c0)
            ps = psum.tile([P, 512], dt.float32, tag="ps", name="ps_r", bufs=4)
            nc.tensor.matmul(ps[:bs, :d_in], lhsT=meanT[:, c0 : c0 + bs], rhs=w1l_sb[:],
                             start=True, stop=False)
            nc.tensor.matmul(ps[:bs, :d_in], lhsT=xT_sb[:, c0 : c0 + bs], rhs=w1r_sb[:],
                             start=False, stop=False)
            nc.tensor.matmul(ps[:bs, :d_in], lhsT=ones_sb[:, :bs], rhs=b1_sb[:],
                             start=False, stop=True)
            hrow = st_p.tile([P, d_in], dt.bfloat16, tag="st", name="hrow")
            nc.scalar.activation(hrow[:bs, :], ps[:bs, :d_in], act.Relu)
            nc.sync.dma_start(hsh[c0 : c0 + bs, :], hrow[:bs, :])

        nc.gpsimd.collective_compute(
            "AllGather", alu.bypass,
            replica_groups=[list(range(m["n_cores"]))],
            ins=[hsh[:].opt()], outs=[hfull[:].opt()],
        )

        for c0 in range(0, rows_per, 512):
            w = min(512, rows_per - c0)
            ps = psum.tile([P, 512], dt.float32, tag="ps", name="ps_d", bufs=4)
            nc.tensor.matmul(ps[:, :w], lhsT=w1l_sb[:], rhs=meanT[:, c0 : c0 + w],
                             start=True, stop=False)
            nc.tensor.matmul(ps[:, :w], lhsT=w1r_sb[:], rhs=xT_sb[:, c0 : c0 + w],
                             start=False, stop=False)
            nc.tensor.matmul(ps[:, :w], lhsT=b1_sb[:], rhs=ones_sb[:, :w],
                             start=False, stop=True)
            nc.scalar.activation(hT[:, c0 : c0 + w], ps[:, :w], act.Relu)

        # ---- layer 2 ----
        aggregate(hfull, meanhT)

        for b in range(nblk):
            c0 = b * P
            bs = min(P, rows_per - c0)
            ps = psum.tile([P, 512], dt.float32, tag="ps", name="ps_o", bufs=4)
            nc.tensor.matmul(ps[:bs, :d_out], lhsT=meanhT[:, c0 : c0 + bs], rhs=w2l_sb[:],
                             start=True, stop=False)
            nc.tensor.matmul(ps[:bs, :d_out], lhsT=hT[:, c0 : c0 + bs], rhs=w2r_sb[:],
                             start=False, stop=False)
            nc.tensor.matmul(ps[:bs, :d_out], lhsT=ones_sb[:, :bs], rhs=b2_sb[:],
                             start=False, stop=True)
            ot = st_p.tile([P, d_out], dt.float32, tag="ot", name="ot")
            nc.vector.tensor_copy(ot[:bs, :], ps[:bs, :d_out])
            nc.sync.dma_start(out_d.ap()[c0 : c0 + bs, :], ot[:bs, :])

    return nc


def make_in_maps(inputs, meta, per_core):
    x = np.asarray(inputs["x"], np.float32)
    n_cores, rows_per = meta["n_cores"], meta["rows_per"]
    x_bf = x.astype(BF16)
    w1l = np.asarray(inputs["W1l"], np.float32)
    w1r = np.asarray(inputs["W1r"], np.float32)
    w2l = np.asarray(inputs["W2l"], np.float32)
    w2r = np.asarray(inputs["W2r"], np.float32)
    b1 = np.asarray(inputs["b1"], np.float32)
    b2 = np.asarray(inputs["b2"], np.float32)
    iota = np.tile(np.arange(P, dtype=np.float32)[None, :], (P, 1)).astype(BF16)
    in_maps = []
    for k in range(n_cores):
        r0 = k * rows_per
        pc = per_core[k]
        in_maps.append({
            "x_all": x_bf,
            "xT": np.ascontiguousarray(x[r0 : r0 + rows_per].T).astype(BF16),
            "idx_lo": pc["idx_lo"], "idx_hi": pc["idx_hi"],
            "dstv_lo": pc["dstv_lo"], "dstv_hi": pc["dstv_hi"],
            "invdeg": pc["invdeg"],
            "iota": iota,
            "w1lT": np.ascontiguousarray(w1l.T).astype(BF16),
            "w1rT": np.ascontiguousarray(w1r.T).astype(BF16),
            "w2lT": np.ascontiguousarray(w2l.T).astype(BF16),
            "w2rT": np.ascontiguousarray(w2r.T).astype(BF16),
            "b1r": b1[None, :].astype(BF16),
            "b2r": b2[None, :].astype(BF16),
        })
    return in_maps


_CACHE = {}


def _compile(meta):
    key = (meta["Llo"], meta["Lhi"], meta["n_nodes"], meta["rows_per"])
    if key not in _CACHE:
        nc = bacc.Bacc("TRN2", target_bir_lowering=False, debug=False,
                       num_devices=meta["n_cores"], num_swdge_queues=NQ)
        build_graph(nc, meta)
        nc.compile()
        _CACHE[key] = nc
    return _CACHE[key]


def kernel(**inputs):
    edge_index = np.asarray(inputs["edge_index"])
    meta, per_core = preprocess(edge_index)
    nc = _compile(meta)
    in_maps = make_in_maps(inputs, meta, per_core)
    res = bass_utils.run_bass_kernel_spmd(
        nc, in_maps, core_ids=list(range(meta["n_cores"]))
    )
    out = np.concatenate(
        [res.results[k]["out"] for k in range(meta["n_cores"])], axis=0
    )
    return out.astype(np.float32)
